# revision 9
# baseline (speedup 1.0000x reference)
"""Trainium2 Bass kernel for nn_ActorCriticGNN_RNN_Policy (GATv2 x2 + GRU + heads).

Self-contained: hardcodes problem shapes (A=16 agents, N=1000 nodes, F=16,
E=16000 edges, GNN_DIM=RNN_DIM=128, 4-head conv1, N_ACT=16), shards by agent
(2 agents per NeuronCore x 8 cores).

Strategy: the graded outputs (logits/values/next_h) depend on h2 only at each
agent's self node, so per core only the dependency cone matters:
  - layer-2 edges: edges into the core's 2 agent nodes (~34)
  - layer-1 edges: edges into the ~36 cone nodes (~650)
The graph structure (edge_index) is known on the host when kernel() is called,
so the cone and all gather/scatter selectors are baked in as small one-hot
matrices and host-expanded feature streams; the device does dense PE matmuls +
DVE/ACT elementwise work. All model arithmetic (matmuls, attention softmax,
LN, GRU, heads) runs on the NeuronCores.
"""

import numpy as np
import ml_dtypes

A, N, F, E = 16, 1000, 16, 16000
GNN, RNN, N_ACT = 128, 128, 16
H1, C1 = 4, 32
NEG = 0.2
NCORES = 8
AGC = A // NCORES  # agents per core = 2

_CACHE = {}


def _build_host_data(edge_index):
    """Per-core index analysis + one-hot selectors. Pure index preprocessing."""
    src_g = edge_index[0].astype(np.int64)
    dst_g = edge_index[1].astype(np.int64)

    cores = []
    nEB_max = 0
    for c in range(NCORES):
        ags = [AGC * c + k for k in range(AGC)]
        mA = np.isin(dst_g, ags)
        eA_src = np.concatenate([src_g[mA], np.array(ags)])
        eA_dst = np.concatenate([dst_g[mA], np.array(ags)])
        nEA = len(eA_src)
        assert nEA <= 128, nEA
        S1 = np.unique(np.concatenate([eA_src, np.array(ags)]))
        m1 = len(S1)
        assert m1 <= 128, m1
        slot = {int(n): i for i, n in enumerate(S1)}
        mB = np.isin(dst_g, S1)
        eB_src = np.concatenate([src_g[mB], S1])
        eB_dst = np.concatenate([dst_g[mB], S1])
        nEB = len(eB_src)
        nEB_max = max(nEB_max, nEB)
        cores.append(dict(ags=ags, eA_src=eA_src, eA_dst=eA_dst, nEA=nEA,
                          S1=S1, m1=m1, slot=slot,
                          eB_src=eB_src, eB_dst=eB_dst, nEB=nEB))
    EBP = 128 * ((nEB_max + 127) // 128)
    for cd in cores:
        cd["EBP"] = EBP
    return cores, EBP


def _pack_layouts(EBP):
    """Column/partition layout of the two per-core constant blobs.

    Each entry: name -> (partition_offset, n_partitions, col_offset, n_cols).
    """
    f32 = {}
    off = 0

    def seq(name, parts, cols):
        nonlocal off
        f32[name] = (0, parts, off, cols)
        off += cols

    # all matmul operands must sit at base partition 0
    seq("nfs0", 16, EBP)
    seq("nfs1", 16, EBP)
    seq("nfd0", 16, EBP)
    seq("nfd1", 16, EBP)
    seq("wl1", 16, 128)
    seq("wr1", 16, 128)
    seq("att1", 128, 4)
    seq("att2c", 128, 1)
    seq("wl2", 128, 128)
    seq("wr2", 128, 128)
    seq("b1bc", 128, 128)
    seq("lnwbc", 128, 128)
    seq("lnbbc", 128, 128)
    seq("idn", 128, 128)
    seq("wih", 128, 384)
    seq("whh", 128, 384)
    seq("rnsT", 128, AGC)
    seq("b2c", 128, 1)
    seq("slotmask", 128, 1)
    seq("slotpad", 128, 1)
    seq("wav", 128, N_ACT + 1)
    seq("bih2", AGC, 384)
    seq("bhh2", AGC, 384)
    seq("rns2", AGC, 128)
    seq("bav", AGC, N_ACT + 1)
    seq("maskf", AGC, N_ACT + 1)
    seq("offs", AGC, N_ACT + 1)
    F32_COLS = off

    bf = {}
    boff = 0

    def bseq(name, parts, cols):
        nonlocal boff
        bf[name] = (0, parts, boff, cols)
        boff += cols

    bseq("d2b", 128, EBP)
    bseq("sa", 128, 128)
    bseq("da", 128, 128)
    bseq("d2a", 128, 128)
    BF_COLS = boff
    return f32, F32_COLS, bf, BF_COLS


def _fill_blobs(cd, inputs, f32lay, F32_COLS, bflay, BF_COLS):
    EBP = cd["EBP"]
    blob = np.zeros((128, F32_COLS), np.float32)
    bblob = np.zeros((128, BF_COLS), np.float32)

    def put(lay, b, name, arr):
        poff, parts, coff, cols = lay[name]
        assert arr.shape == (parts, cols), (name, arr.shape, (parts, cols))
        b[poff:poff + parts, coff:coff + cols] = arr

    nf = inputs["node_features"]
    ags = cd["ags"]
    nEB = cd["nEB"]
    for k in range(AGC):
        a = ags[k]
        s = np.zeros((16, EBP), np.float32)
        d = np.zeros((16, EBP), np.float32)
        s[:, :nEB] = nf[a][cd["eB_src"]].T
        d[:, :nEB] = nf[a][cd["eB_dst"]].T
        put(f32lay, blob, f"nfs{k}", s)
        put(f32lay, blob, f"nfd{k}", d)

    put(f32lay, blob, "wl1", inputs["Wl1"])
    put(f32lay, blob, "wr1", inputs["Wr1"])
    att1 = inputs["att1"]  # [4, 32]
    ab = np.zeros((128, 4), np.float32)
    for h in range(H1):
        ab[h * C1:(h + 1) * C1, h] = att1[h]
    put(f32lay, blob, "att1", ab)
    put(f32lay, blob, "att2c", inputs["att2"].reshape(128, 1))
    put(f32lay, blob, "wl2", inputs["Wl2"])
    put(f32lay, blob, "wr2", inputs["Wr2"])
    put(f32lay, blob, "b1bc", np.tile(inputs["b1"][None, :], (128, 1)))
    put(f32lay, blob, "lnwbc", np.tile(inputs["ln_w"][None, :], (128, 1)))
    put(f32lay, blob, "lnbbc", np.tile(inputs["ln_b"][None, :], (128, 1)))
    put(f32lay, blob, "idn", np.eye(128, dtype=np.float32))
    put(f32lay, blob, "wih", inputs["W_ih"].T.astype(np.float32))
    put(f32lay, blob, "whh", inputs["W_hh"].T.astype(np.float32))
    put(f32lay, blob, "rnsT", inputs["rnn_state"][ags].T.astype(np.float32))
    put(f32lay, blob, "b2c", inputs["b2"].reshape(128, 1))
    sm = np.zeros((128, 1), np.float32)
    sm[:cd["m1"], 0] = 1.0
    put(f32lay, blob, "slotmask", sm)
    put(f32lay, blob, "slotpad", (1.0 - sm))
    put(f32lay, blob, "wav",
        np.concatenate([inputs["Wa"], inputs["Wv"]], axis=1).astype(np.float32))
    put(f32lay, blob, "bih2", np.tile(inputs["b_ih"][None, :], (AGC, 1)))
    put(f32lay, blob, "bhh2", np.tile(inputs["b_hh"][None, :], (AGC, 1)))
    put(f32lay, blob, "rns2", inputs["rnn_state"][ags].astype(np.float32))
    bav = np.concatenate([inputs["ba"], inputs["bv"]])[None, :]
    put(f32lay, blob, "bav", np.tile(bav, (AGC, 1)).astype(np.float32))
    m = inputs["action_mask"][ags].astype(np.float32)
    mf = np.concatenate([m, np.ones((AGC, 1), np.float32)], axis=1)
    put(f32lay, blob, "maskf", mf)
    offs = np.concatenate([(1.0 - m) * np.float32(-1e8),
                           np.zeros((AGC, 1), np.float32)], axis=1)
    put(f32lay, blob, "offs", offs)

    # bf16 one-hots
    slot = cd["slot"]
    d2b = np.zeros((128, EBP // 128, 128), np.float32)
    for e in range(nEB):
        d2b[e % 128, e // 128, slot[int(cd["eB_dst"][e])]] = 1.0
    put(bflay, bblob, "d2b", d2b.reshape(128, EBP))
    sa = np.zeros((128, 128), np.float32)
    da = np.zeros((128, 128), np.float32)
    d2a = np.zeros((128, 128), np.float32)
    for e in range(cd["nEA"]):
        sa[slot[int(cd["eA_src"][e])], e] = 1.0
        da[slot[int(cd["eA_dst"][e])], e] = 1.0
        k = cd["ags"].index(int(cd["eA_dst"][e]))
        d2a[e, k] = 1.0
    put(bflay, bblob, "sa", sa)
    put(bflay, bblob, "da", da)
    put(bflay, bblob, "d2a", d2a)

    return blob, bblob.astype(ml_dtypes.bfloat16)


def _build_nc(EBP, f32lay, F32_COLS, bflay, BF_COLS, reps=1):
    import concourse.bacc as bacc
    import concourse.mybir as mybir
    import concourse.tile as tile

    F32 = mybir.dt.float32
    BF16 = mybir.dt.bfloat16
    AF = mybir.ActivationFunctionType
    OP = mybir.AluOpType
    AX = mybir.AxisListType

    nc = bacc.Bacc("TRN2", target_bir_lowering=False, debug=False)
    blob_d = nc.dram_tensor("blob", [128, F32_COLS], F32, kind="ExternalInput")
    bblob_d = nc.dram_tensor("bblob", [128, BF_COLS], BF16, kind="ExternalInput")
    lo_d = nc.dram_tensor("out_lo", [AGC, N_ACT], F32, kind="ExternalOutput")
    v_d = nc.dram_tensor("out_v", [AGC, 1], F32, kind="ExternalOutput")
    h_d = nc.dram_tensor("out_h", [AGC, RNN], F32, kind="ExternalOutput")

    NSUB = EBP // 128

    with tile.TileContext(nc) as tc:
        with (
            tc.tile_pool(name="const", bufs=1) as cp,
            tc.tile_pool(name="work", bufs=2) as wp,
            tc.tile_pool(name="pz", bufs=1, space="PSUM") as pz,
            tc.tile_pool(name="pacc", bufs=1, space="PSUM") as pacc,
            tc.tile_pool(name="pt", bufs=3, space="PSUM") as pt,
        ):
            for rep in range(reps):
                blob = cp.tile([128, F32_COLS], F32, tag="blob")
                nc.sync.dma_start(out=blob[:], in_=blob_d[:])
                bblob = cp.tile([128, BF_COLS], BF16, tag="bblob")
                nc.sync.dma_start(out=bblob[:], in_=bblob_d[:])

                def g(name):
                    lay, t = (f32lay, blob) if name in f32lay else (bflay, bblob)
                    poff, parts, coff, cols = lay[name]
                    return t[poff:poff + parts, coff:coff + cols]

                ones1 = cp.tile([1, 128], F32, tag="ones1")
                nc.vector.memset(ones1[:], 1.0)

                afT = wp.tile([128, AGC], F32, tag="afT")

                for k in range(AGC):
                    nfs = g(f"nfs{k}")
                    nfd = g(f"nfd{k}")

                    # ---- layer 1 on EB edges ----
                    # z1 [C, e] C-major in PSUM
                    z1 = pz.tile([128, EBP], F32, tag="z1")
                    for j0 in range(0, EBP, 512):
                        j1 = min(j0 + 512, EBP)
                        nc.tensor.matmul(z1[:, j0:j1], lhsT=g("wl1"),
                                         rhs=nfs[:, j0:j1], start=True, stop=False)
                        nc.tensor.matmul(z1[:, j0:j1], lhsT=g("wr1"),
                                         rhs=nfd[:, j0:j1], start=False, stop=True)
                    # t1 = lrelu(z1) bf16
                    t1a = wp.tile([128, EBP], F32, tag="t1a")
                    nc.vector.tensor_scalar(out=t1a[:], in0=z1[:], scalar1=NEG,
                                            scalar2=None, op0=OP.mult)
                    t1 = wp.tile([128, EBP], F32, tag="t1")
                    nc.vector.tensor_tensor(out=t1[:], in0=t1a[:], in1=z1[:],
                                            op=OP.max)
                    # e1/exp per subchunk; transpose each into aT1 column slices
                    aT1 = wp.tile([128, 4 * NSUB], F32, tag="aT1")
                    aT1b = wp.tile([128, 4 * NSUB], BF16, tag="aT1b")
                    for s in range(NSUB):
                        e1s = pacc.tile([4, 128], F32, tag="e1")
                        nc.tensor.matmul(e1s[:], lhsT=g("att1"),
                                         rhs=t1[:, 128 * s:128 * (s + 1)],
                                         start=True, stop=True)
                        a1s = wp.tile([4, 128], F32, tag="a1s")
                        nc.scalar.activation(a1s[:], e1s[:], AF.Exp)
                        aT1p = pt.tile([128, 4], F32, tag="tp")
                        nc.tensor.transpose(aT1p[:], a1s[:], g("idn")[:4, :4])
                        nc.vector.tensor_copy(aT1[:, 4 * s:4 * s + 4], aT1p[:])
                        nc.vector.tensor_copy(aT1b[:, 4 * s:4 * s + 4], aT1p[:])

                    # glw per subchunk; u1/s1 accumulate
                    u1 = pacc.tile([128, 128], F32, tag="u1")
                    s1 = pacc.tile([4, 128], F32, tag="s1")
                    for s in range(NSUB):
                        glr = pt.tile([128, 128], F32, tag="tp")
                        nc.tensor.matmul(glr[:], lhsT=nfs[:, 128 * s:128 * (s + 1)],
                                         rhs=g("wl1"), start=True, stop=True)
                        glw = wp.tile([128, 128], BF16, tag="glw")
                        for h in range(H1):
                            nc.vector.tensor_scalar(
                                out=glw[:, C1 * h:C1 * (h + 1)],
                                in0=glr[:, C1 * h:C1 * (h + 1)],
                                scalar1=aT1[:, 4 * s + h:4 * s + h + 1],
                                scalar2=None, op0=OP.mult)
                        d2b_s = g("d2b").rearrange("p (s q) -> p s q", q=128)[:, s, :]
                        nc.tensor.matmul(u1[:], lhsT=glw[:], rhs=d2b_s,
                                         start=(s == 0), stop=(s == NSUB - 1))
                        nc.tensor.matmul(s1[:], lhsT=aT1b[:, 4 * s:4 * s + 4],
                                         rhs=d2b_s,
                                         start=(s == 0), stop=(s == NSUB - 1))

                    # h_mini: transpose u1 -> [slot, hc]; divide by s1; +b1; LN; relu
                    u1s = wp.tile([128, 128], F32, tag="u1s")
                    nc.vector.tensor_copy(u1s[:], u1[:])
                    hnp = pt.tile([128, 128], F32, tag="tp")
                    nc.tensor.transpose(hnp[:], u1s[:], g("idn"))
                    s1s = wp.tile([4, 128], F32, tag="s1s")
                    nc.vector.tensor_copy(s1s[:], s1[:])
                    s1Tp = pt.tile([128, 4], F32, tag="tp")
                    nc.tensor.transpose(s1Tp[:], s1s[:], g("idn")[:4, :4])
                    s1T = wp.tile([128, 4], F32, tag="s1T")
                    nc.vector.tensor_copy(s1T[:], s1Tp[:])
                    nc.vector.tensor_tensor(
                        out=s1T[:].rearrange("p (a b) -> p a b", b=4),
                        in0=s1T[:].rearrange("p (a b) -> p a b", b=4),
                        in1=g("slotpad").rearrange("p (a b) -> p a b", b=1)
                            .broadcast_to([128, 1, 4]),
                        op=OP.add)
                    s1r = wp.tile([128, 4], F32, tag="s1r")
                    nc.vector.reciprocal(s1r[:], s1T[:])
                    hm = wp.tile([128, 128], F32, tag="hm")
                    nc.vector.tensor_tensor(
                        out=hm[:].rearrange("p (a b) -> p a b", a=H1),
                        in0=hnp[:].rearrange("p (a b) -> p a b", a=H1),
                        in1=s1r[:].rearrange("p (a b) -> p a b", b=1)
                            .broadcast_to([128, H1, C1]),
                        op=OP.mult)
                    nc.vector.tensor_tensor(out=hm[:], in0=hm[:], in1=g("b1bc"),
                                            op=OP.add)
                    # LayerNorm over hc (free axis), then relu, then slot mask
                    red = wp.tile([128, 1], F32, tag="red")
                    nc.vector.tensor_reduce(out=red[:], in_=hm[:], op=OP.add, axis=AX.X)
                    mu = wp.tile([128, 1], F32, tag="mu")
                    nc.vector.tensor_scalar(out=mu[:], in0=red[:], scalar1=1.0 / 128,
                                            scalar2=None, op0=OP.mult)
                    nc.vector.tensor_scalar(out=hm[:], in0=hm[:], scalar1=mu[:],
                                            scalar2=None, op0=OP.subtract)
                    sq = wp.tile([128, 128], F32, tag="sq")
                    nc.vector.tensor_tensor(out=sq[:], in0=hm[:], in1=hm[:], op=OP.mult)
                    nc.vector.tensor_reduce(out=red[:], in_=sq[:], op=OP.add, axis=AX.X)
                    var = wp.tile([128, 1], F32, tag="var")
                    nc.vector.tensor_scalar(out=var[:], in0=red[:], scalar1=1.0 / 128,
                                            scalar2=1e-5, op0=OP.mult, op1=OP.add)
                    rec = wp.tile([128, 1], F32, tag="rec")
                    nc.vector.reciprocal(rec[:], var[:])
                    rstd = wp.tile([128, 1], F32, tag="rstd")
                    nc.scalar.activation(rstd[:], rec[:], AF.Sqrt)
                    nc.vector.tensor_scalar(out=hm[:], in0=hm[:], scalar1=rstd[:],
                                            scalar2=None, op0=OP.mult)
                    nc.vector.tensor_tensor(out=hm[:], in0=hm[:], in1=g("lnwbc"),
                                            op=OP.mult)
                    nc.vector.tensor_tensor(out=hm[:], in0=hm[:], in1=g("lnbbc"),
                                            op=OP.add)
                    nc.vector.tensor_scalar(out=hm[:], in0=hm[:], scalar1=0.0,
                                            scalar2=None, op0=OP.max)
                    nc.vector.tensor_scalar(out=hm[:], in0=hm[:],
                                            scalar1=g("slotmask"),
                                            scalar2=None, op0=OP.mult)

                    # ---- layer 2 on EA edges ----
                    hmTp = pt.tile([128, 128], F32, tag="tp")
                    nc.tensor.transpose(hmTp[:], hm[:], g("idn"))
                    hmT = wp.tile([128, 128], F32, tag="hmT")
                    nc.vector.tensor_copy(hmT[:], hmTp[:])
                    xl2p = pt.tile([128, 128], F32, tag="tp")
                    nc.tensor.matmul(xl2p[:], lhsT=hmT[:], rhs=g("wl2"),
                                     start=True, stop=True)
                    xl2 = wp.tile([128, 128], BF16, tag="xl2")
                    nc.vector.tensor_copy(xl2[:], xl2p[:])
                    xr2p = pt.tile([128, 128], F32, tag="tp")
                    nc.tensor.matmul(xr2p[:], lhsT=hmT[:], rhs=g("wr2"),
                                     start=True, stop=True)
                    xr2 = wp.tile([128, 128], BF16, tag="xr2")
                    nc.vector.tensor_copy(xr2[:], xr2p[:])

                    z2 = pt.tile([128, 128], F32, tag="tp")
                    nc.tensor.matmul(z2[:], lhsT=xl2[:], rhs=g("sa"),
                                     start=True, stop=False)
                    nc.tensor.matmul(z2[:], lhsT=xr2[:], rhs=g("da"),
                                     start=False, stop=True)
                    t2a = wp.tile([128, 128], F32, tag="t2a")
                    nc.vector.tensor_scalar(out=t2a[:], in0=z2[:], scalar1=NEG,
                                            scalar2=None, op0=OP.mult)
                    t2 = wp.tile([128, 128], F32, tag="t2")
                    nc.vector.tensor_tensor(out=t2[:], in0=t2a[:], in1=z2[:],
                                            op=OP.max)
                    e2 = pt.tile([1, 128], F32, tag="tp")
                    nc.tensor.matmul(e2[:], lhsT=g("att2c"), rhs=t2[:],
                                     start=True, stop=True)
                    a2 = wp.tile([1, 128], F32, tag="a2")
                    nc.scalar.activation(a2[:], e2[:], AF.Exp)
                    a2Tp = pt.tile([128, 1], F32, tag="tp")
                    nc.tensor.transpose(a2Tp[:], a2[:], g("idn")[:1, :1])
                    a2T = wp.tile([128, 1], F32, tag="a2T")
                    nc.vector.tensor_copy(a2T[:], a2Tp[:])
                    a2Tb = wp.tile([128, 1], BF16, tag="a2Tb")
                    nc.vector.tensor_copy(a2Tb[:], a2Tp[:])

                    glr2 = pt.tile([128, 128], F32, tag="tp")
                    nc.tensor.matmul(glr2[:], lhsT=g("sa"), rhs=xl2[:],
                                     start=True, stop=True)
                    glw2 = wp.tile([128, 128], BF16, tag="glw2")
                    nc.vector.tensor_scalar(out=glw2[:], in0=glr2[:],
                                            scalar1=a2T[:], scalar2=None,
                                            op0=OP.mult)
                    u2 = pt.tile([128, 128], F32, tag="tp")
                    nc.tensor.matmul(u2[:], lhsT=glw2[:], rhs=g("d2a"),
                                     start=True, stop=True)
                    s2 = pt.tile([1, 128], F32, tag="tp")
                    nc.tensor.matmul(s2[:], lhsT=a2Tb[:], rhs=g("d2a"),
                                     start=True, stop=True)

                    s2s = wp.tile([1, AGC], F32, tag="s2s")
                    nc.vector.tensor_copy(s2s[:], s2[0:1, 0:AGC])
                    s2r = wp.tile([1, AGC], F32, tag="s2r")
                    nc.vector.reciprocal(s2r[:], s2s[:])
                    s2bp = pt.tile([128, AGC], F32, tag="tp")
                    nc.tensor.matmul(s2bp[:], lhsT=ones1[:], rhs=s2r[:],
                                     start=True, stop=True)
                    s2b = wp.tile([128, AGC], F32, tag="s2b")
                    nc.vector.tensor_copy(s2b[:], s2bp[:])
                    af1 = wp.tile([128, 1], F32, tag="af1")
                    nc.vector.tensor_tensor(out=af1[:], in0=u2[:, k:k + 1],
                                            in1=s2b[:, k:k + 1], op=OP.mult)
                    nc.vector.tensor_scalar(out=afT[:, k:k + 1], in0=af1[:],
                                            scalar1=g("b2c"), scalar2=None,
                                            op0=OP.add)

                # ---- GRU + heads (both agents batched) ----
                gi = pt.tile([AGC, 3 * RNN], F32, tag="tp")
                nc.tensor.matmul(gi[:], lhsT=afT[:], rhs=g("wih"),
                                 start=True, stop=True)
                gh = pt.tile([AGC, 3 * RNN], F32, tag="tp")
                nc.tensor.matmul(gh[:], lhsT=g("rnsT"), rhs=g("whh"),
                                 start=True, stop=True)
                gis = wp.tile([AGC, 3 * RNN], F32, tag="gis")
                nc.vector.tensor_tensor(out=gis[:], in0=gi[:], in1=g("bih2"), op=OP.add)
                ghs = wp.tile([AGC, 3 * RNN], F32, tag="ghs")
                nc.vector.tensor_tensor(out=ghs[:], in0=gh[:], in1=g("bhh2"), op=OP.add)

                tmp = wp.tile([AGC, RNN], F32, tag="tmp")
                nc.vector.tensor_tensor(out=tmp[:], in0=gis[:, 0:RNN],
                                        in1=ghs[:, 0:RNN], op=OP.add)
                r = wp.tile([AGC, RNN], F32, tag="r")
                nc.scalar.activation(r[:], tmp[:], AF.Sigmoid)
                nc.vector.tensor_tensor(out=tmp[:], in0=gis[:, RNN:2 * RNN],
                                        in1=ghs[:, RNN:2 * RNN], op=OP.add)
                zg = wp.tile([AGC, RNN], F32, tag="zg")
                nc.scalar.activation(zg[:], tmp[:], AF.Sigmoid)
                nc.vector.tensor_tensor(out=tmp[:], in0=r[:],
                                        in1=ghs[:, 2 * RNN:], op=OP.mult)
                nc.vector.tensor_tensor(out=tmp[:], in0=tmp[:],
                                        in1=gis[:, 2 * RNN:], op=OP.add)
                n_t = wp.tile([AGC, RNN], F32, tag="n_t")
                nc.scalar.activation(n_t[:], tmp[:], AF.Tanh)
                # h' = (1-z)*n + z*h = n - z*n + z*h
                zn = wp.tile([AGC, RNN], F32, tag="zn")
                nc.vector.tensor_tensor(out=zn[:], in0=zg[:], in1=n_t[:], op=OP.mult)
                zh = wp.tile([AGC, RNN], F32, tag="zh")
                nc.vector.tensor_tensor(out=zh[:], in0=zg[:], in1=g("rns2"), op=OP.mult)
                hnew = wp.tile([AGC, RNN], F32, tag="hnew")
                nc.vector.tensor_tensor(out=hnew[:], in0=n_t[:], in1=zn[:],
                                        op=OP.subtract)
                nc.vector.tensor_tensor(out=hnew[:], in0=hnew[:], in1=zh[:], op=OP.add)

                hnTp = pt.tile([128, AGC], F32, tag="tp")
                nc.tensor.transpose(hnTp[:], hnew[:], g("idn")[:AGC, :AGC])
                hnT = wp.tile([128, AGC], F32, tag="hnT")
                nc.vector.tensor_copy(hnT[:], hnTp[:])
                lo = pt.tile([AGC, N_ACT + 1], F32, tag="tp")
                nc.tensor.matmul(lo[:], lhsT=hnT[:], rhs=g("wav"),
                                 start=True, stop=True)
                los = wp.tile([AGC, N_ACT + 1], F32, tag="los")
                nc.vector.tensor_tensor(out=los[:], in0=lo[:], in1=g("bav"), op=OP.add)
                nc.vector.tensor_tensor(out=los[:], in0=los[:], in1=g("maskf"),
                                        op=OP.mult)
                nc.vector.tensor_tensor(out=los[:], in0=los[:], in1=g("offs"),
                                        op=OP.add)

                nc.sync.dma_start(out=lo_d[:], in_=los[:, 0:N_ACT])
                nc.sync.dma_start(out=v_d[:], in_=los[:, N_ACT:N_ACT + 1])
                nc.sync.dma_start(out=h_d[:], in_=hnew[:])

    nc.compile()
    return nc


def _get_compiled(edge_index, reps=1):
    key = (edge_index.tobytes(), reps)
    if key in _CACHE:
        return _CACHE[key]
    cores, EBP = _build_host_data(edge_index)
    f32lay, F32_COLS, bflay, BF_COLS = _pack_layouts(EBP)
    nc = _build_nc(EBP, f32lay, F32_COLS, bflay, BF_COLS, reps=reps)
    _CACHE[key] = (cores, f32lay, F32_COLS, bflay, BF_COLS, nc)
    return _CACHE[key]


def kernel(reps=1, **inputs):
    from concourse.bass_utils import run_bass_kernel_spmd

    edge_index = np.asarray(inputs["edge_index"])
    cores, f32lay, F32_COLS, bflay, BF_COLS, nc = _get_compiled(edge_index, reps)

    in_maps = []
    for c in range(NCORES):
        blob, bblob = _fill_blobs(cores[c], inputs, f32lay, F32_COLS,
                                  bflay, BF_COLS)
        in_maps.append({"blob": blob, "bblob": bblob})

    res = run_bass_kernel_spmd(nc, in_maps, list(range(NCORES)))
    logits = np.zeros((A, N_ACT), np.float32)
    values = np.zeros((A, 1), np.float32)
    next_h = np.zeros((A, RNN), np.float32)
    for c in range(NCORES):
        r = res.results[c]
        logits[AGC * c:AGC * (c + 1)] = r["out_lo"]
        values[AGC * c:AGC * (c + 1)] = r["out_v"]
        next_h[AGC * c:AGC * (c + 1)] = r["out_h"]
    return logits, values, next_h
